# revision 1
# baseline (speedup 1.0000x reference)
"""Trainium2 Bass kernel for the GCN model (nn_GCNModel_57853209477141).

Model: 3x GCNConv(128->128, sym-norm with self loops) with ReLU, question
embedding MLP, concat, 2-layer MLP head -> [50000, 32].

Strategy (8 NeuronCores, single SPMD launch):
- dst-node sharding: global tiles of 128 nodes; tile t -> core t % 8; within a
  core, tiles sorted by edge count desc -> slot order, so one compile-time
  chunks-per-slot schedule serves all 8 cores (SPMD = one program).
- GCN norm factorization: agg[v] = dinv[v] * sum_{e->v} (dinv*h)[src_e], so
  the gather table stores h~ = dinv * h and the per-edge norm disappears.
- aggregation: per slot (= one 128-dst-node tile), edges in chunks of 128;
  per chunk gather 128 table rows (indirect DMA), build a 0/1 one-hot
  [edge, dst] via iota-compare on DVE, matmul-accumulate one-hot^T @ gathered
  into PSUM.  Chunk padding edges carry dstin=-1 -> all-zero one-hot row.
- between layers: next h~ slice is produced in the slot epilogue
  (relu(dinv*agg+b) -> PE transpose -> matmul W_next -> scale dinv) and
  AllGather replicates the new table into every core's DRAM.
- question path: qq = relu(qe@fc0+b)@fc1[128:] + fc1_b computed once, per-slot
  rows gathered by graph id; out = relu(h3@fc1[:128] + qq_g) @ fc2 + fc2_b.

Host preprocessing is index work only (sharding, edge sort, degree counts);
all O(E*F) / O(N*F*F) float work runs on device.
"""
import os
import sys
import types
from contextlib import ExitStack

import numpy as np

# ---------------------------------------------------------------- constants
N = 50000
E = 800000
G = 64
P = 128
NCORES = 8
TPC = 49  # tile slots per core
SLOT_ROWS = TPC * P  # 6272
NT = NCORES * SLOT_ROWS  # 50176
QD = 768
OUTC = 32

AGG_DT = os.environ.get("GCN_AGG_DT", "f32r")  # f32 | f32r | bf16
BF16 = AGG_DT == "bf16"
NBLK = 7  # AllGather split blocks (layers 1-2)
SPB = TPC // NBLK  # slots per block
SPLIT0 = 25  # layer-0 AllGather: slots [0,SPLIT0) in first half
AG_SPLIT = int(os.environ.get("GCN_AG_SPLIT", "1"))  # row blocks per AllGather
assert TPC % AG_SPLIT == 0 or AG_SPLIT == 1


def _install_axon_prof():
    """Register NTFF profile hook if the image's antenv lacks it; neuter
    bucket upload (zero-egress). Harmless when running without tracing."""
    try:
        from antenv import axon_hooks  # noqa: F401
    except ImportError:
        try:
            import antenv
            from trn_agent_boot.trn_boot import _ntff_profile_via_ctypes

            hook = _ntff_profile_via_ctypes("/opt/axon/libaxon_pjrt.so")
            mod = types.ModuleType("antenv.axon_hooks")
            mod.get_axon_ntff_profile_hook = lambda: hook
            mod.set_axon_ntff_profile_hook = lambda h: None
            sys.modules["antenv.axon_hooks"] = mod
            antenv.axon_hooks = mod
        except Exception:
            pass
    try:
        import concourse.bass_utils as bu

        bu.upload_artifacts = lambda tmpdir: "local://" + str(tmpdir)
    except Exception:
        pass


# ---------------------------------------------------------------- host prep
def preprocess(edge_index, batch):
    src = np.asarray(edge_index[0], dtype=np.int64)
    dst = np.asarray(edge_index[1], dtype=np.int64)
    loop = np.arange(N, dtype=np.int64)
    # degree includes self-loops (GCN norm); the self-loop term itself is
    # added on-device from the resident h~ slice, not gathered.
    deg = (np.bincount(dst, minlength=N) + 1).astype(np.float64)
    dinv = np.where(deg > 0, 1.0 / np.sqrt(deg), 0.0).astype(np.float32)
    src_all = src
    dst_all = dst

    n_tiles = (N + P - 1) // P  # 391
    tile_of_node = np.arange(N) // P

    dst_tile = dst_all // P
    tile_counts = np.bincount(dst_tile, minlength=n_tiles)

    # snake-deal tiles (sorted by edge count desc) across cores so every
    # slot's per-core counts are nearly equal -> minimal chunk padding
    order_all = np.argsort(-tile_counts, kind="stable")
    core_tiles = [[] for _ in range(NCORES)]
    for r in range(TPC):
        batch_t = order_all[r * NCORES : (r + 1) * NCORES]
        seq = range(NCORES) if r % 2 == 0 else range(NCORES - 1, -1, -1)
        for j, c in enumerate(seq):
            core_tiles[c].append(int(batch_t[j]) if j < len(batch_t) else -1)
    core_of_tile = np.full(n_tiles, -1, dtype=np.int64)
    for c in range(NCORES):
        for t in core_tiles[c]:
            if t >= 0:
                core_of_tile[t] = c

    chunks = np.zeros(TPC, dtype=np.int64)
    for s in range(TPC):
        for c in range(NCORES):
            t = core_tiles[c][s]
            cnt = int(tile_counts[t]) if t >= 0 else 0
            chunks[s] = max(chunks[s], (cnt + P - 1) // P)
    chunks = np.maximum(chunks, 1)
    total_chunks = int(chunks.sum())

    slot_of_tile = np.full(n_tiles, -1, dtype=np.int64)
    for c in range(NCORES):
        for s, t in enumerate(core_tiles[c]):
            if t >= 0:
                slot_of_tile[t] = s
    # layer-0 table: rank-major (one whole-table AllGather after production).
    table_row0 = (
        core_of_tile[tile_of_node] * SLOT_ROWS
        + slot_of_tile[tile_of_node] * P
        + (np.arange(N) % P)
    )
    # layers 1-2: block-major layout; AllGather is split into NBLK row-blocks
    # fired from the epilogues so collective traffic overlaps aggregation.
    blk = slot_of_tile[tile_of_node] // SPB
    table_rowB = (
        blk * (NCORES * SPB * P)
        + core_of_tile[tile_of_node] * (SPB * P)
        + (slot_of_tile[tile_of_node] % SPB) * P
        + (np.arange(N) % P)
    )

    order = np.argsort(dst_tile, kind="stable")
    src_sorted = src_all[order]
    dst_sorted = dst_all[order]
    sorted_tiles = dst_tile[order]
    tile_starts = np.searchsorted(sorted_tiles, np.arange(n_tiles))
    tile_ends = np.searchsorted(sorted_tiles, np.arange(n_tiles), side="right")

    src_T0 = np.zeros((NCORES, P, total_chunks), dtype=np.int32)
    src_TB = np.zeros((NCORES, P, total_chunks), dtype=np.int32)
    dstin_T = np.full((NCORES, P, total_chunks), -1.0, dtype=np.float32)
    chunk_base = np.cumsum(np.concatenate([[0], chunks[:-1]]))
    for c in range(NCORES):
        for s in range(TPC):
            t = core_tiles[c][s]
            if t < 0:
                continue
            lo, hi = tile_starts[t], tile_ends[t]
            cnt = hi - lo
            nchunk = int(chunks[s])
            buf0 = np.zeros(nchunk * P, dtype=np.int32)
            bufB = np.zeros(nchunk * P, dtype=np.int32)
            buf_dst = np.full(nchunk * P, -1.0, dtype=np.float32)
            buf0[:cnt] = table_row0[src_sorted[lo:hi]]
            bufB[:cnt] = table_rowB[src_sorted[lo:hi]]
            buf_dst[:cnt] = (dst_sorted[lo:hi] % P).astype(np.float32)
            b = chunk_base[s]
            src_T0[c, :, b : b + nchunk] = buf0.reshape(nchunk, P).T
            src_TB[c, :, b : b + nchunk] = bufB.reshape(nchunk, P).T
            dstin_T[c, :, b : b + nchunk] = buf_dst.reshape(nchunk, P).T

    dinv_slot = np.zeros((NCORES, P, TPC), dtype=np.float32)
    gid_slot = np.zeros((NCORES, P, TPC), dtype=np.int32)
    node_perm = np.full((NCORES, SLOT_ROWS), -1, dtype=np.int64)
    batch = np.asarray(batch, dtype=np.int64)
    for c in range(NCORES):
        for s in range(TPC):
            t = core_tiles[c][s]
            if t < 0:
                continue
            v0 = t * P
            v1 = min(v0 + P, N)
            n = v1 - v0
            dinv_slot[c, :n, s] = dinv[v0:v1]
            gid_slot[c, :n, s] = batch[v0:v1]
            node_perm[c, s * P : s * P + n] = np.arange(v0, v1)

    return dict(
        chunks=chunks,
        total_chunks=total_chunks,
        src_T0=src_T0,
        src_TB=src_TB,
        dstin_T=dstin_T,
        dinv_slot=dinv_slot,
        gid_slot=gid_slot,
        node_perm=node_perm,
    )


# ------------------------------------------------------------- bass program
def build_program(chunks):
    import concourse.bacc as bacc
    import concourse.bass as bass
    import concourse.tile as tile
    from concourse import mybir
    from concourse.masks import make_identity

    FDT = {"f32": mybir.dt.float32, "f32r": mybir.dt.float32r, "bf16": mybir.dt.bfloat16}[AGG_DT]
    F32 = mybir.dt.float32
    I32 = mybir.dt.int32
    TCH = int(np.sum(chunks))

    nc = bacc.Bacc("TRN2", target_bir_lowering=False)
    dp = nc.declare_dram_parameter
    xT = dp("xT", [P, SLOT_ROWS], F32, isOutput=False)
    idx0 = dp("idx0", [P, TCH], I32, isOutput=False)
    idxB = dp("idxB", [P, TCH], I32, isOutput=False)
    dstin = dp("dstin", [P, TCH], FDT, isOutput=False)
    iota_in = dp("iota_in", [P, P], FDT, isOutput=False)
    dinv_in = dp("dinv_in", [P, TPC], F32, isOutput=False)
    gid_in = dp("gid_in", [P, TPC], I32, isOutput=False)
    W_in = [dp(f"W{i}", [P, P], F32, isOutput=False) for i in range(3)]
    bb_in = [dp(f"bb{i}", [P, P], F32, isOutput=False) for i in range(3)]
    qeT_in = dp("qeT", [QD, G], F32, isOutput=False)
    fc0w_in = dp("fc0w", [QD, P], F32, isOutput=False)
    fc0bb_in = dp("fc0bb", [P, P], F32, isOutput=False)
    fc1a_in = dp("fc1a", [P, P], F32, isOutput=False)
    fc1b_in = dp("fc1b", [P, P], F32, isOutput=False)
    fc1bb_in = dp("fc1bb", [P, P], F32, isOutput=False)
    fc2w_in = dp("fc2w", [P, OUTC], F32, isOutput=False)
    fc2bb_in = dp("fc2bb", [P, OUTC], F32, isOutput=False)
    out_d = dp("out", [SLOT_ROWS, OUTC], F32, isOutput=True)

    cc_in = nc.dram_tensor("cc_in", [SLOT_ROWS, P], FDT)
    tables = [
        nc.dram_tensor(f"table{l}", [NT, P], FDT, addr_space="Shared")
        for l in range(3)
    ]
    qq_d = nc.dram_tensor("qq_d", [G, P], F32)

    chunk_base = np.cumsum(np.concatenate([[0], chunks[:-1]])).astype(int)

    with tile.TileContext(nc) as tc, ExitStack() as ctx:
        const = ctx.enter_context(tc.tile_pool(name="const", bufs=1))
        gp = ctx.enter_context(tc.tile_pool(name="gp", bufs=16))
        ohp = ctx.enter_context(tc.tile_pool(name="ohp", bufs=16))
        psp = ctx.enter_context(tc.tile_pool(name="psp", bufs=3, space="PSUM"))
        psagg = ctx.enter_context(tc.tile_pool(name="psagg", bufs=3, space="PSUM"))
        pst = ctx.enter_context(tc.tile_pool(name="pst", bufs=2, space="PSUM"))
        epi = ctx.enter_context(tc.tile_pool(name="epi", bufs=3))
        xp = ctx.enter_context(tc.tile_pool(name="xp", bufs=3))

        # ---- constants (xT + W0 first: they gate layer-0 production,
        # which gates the startup AllGather)
        xT_sb = const.tile([P, SLOT_ROWS], F32)
        nc.sync.dma_start(out=xT_sb[:], in_=xT[:])
        iota_sb = const.tile([P, P], FDT)
        nc.sync.dma_start(out=iota_sb[:], in_=iota_in[:])
        idx0_sb = const.tile([P, TCH], I32)
        nc.scalar.dma_start(out=idx0_sb[:], in_=idx0[:])
        idxB_sb = const.tile([P, TCH], I32)
        nc.scalar.dma_start(out=idxB_sb[:], in_=idxB[:])
        dstin_sb = const.tile([P, TCH], FDT)
        nc.scalar.dma_start(out=dstin_sb[:], in_=dstin[:])
        dinv_sb = const.tile([P, TPC], F32)
        nc.sync.dma_start(out=dinv_sb[:], in_=dinv_in[:])
        gid_sb = const.tile([P, TPC], I32)
        nc.sync.dma_start(out=gid_sb[:], in_=gid_in[:])
        W_sb = []
        for i in range(3):
            w = const.tile([P, P], F32, tag=f"W{i}")
            nc.sync.dma_start(out=w[:], in_=W_in[i][:])
            W_sb.append(w)
        bb_sb = []
        for i in range(3):
            b = const.tile([P, P], F32, tag=f"bb{i}")
            nc.sync.dma_start(out=b[:], in_=bb_in[i][:])
            bb_sb.append(b)
        fc1a_sb = const.tile([P, P], F32)
        nc.sync.dma_start(out=fc1a_sb[:], in_=fc1a_in[:])
        fc2w_sb = const.tile([P, OUTC], F32)
        nc.sync.dma_start(out=fc2w_sb[:], in_=fc2w_in[:])
        fc2bb_sb = const.tile([P, OUTC], F32)
        nc.sync.dma_start(out=fc2bb_sb[:], in_=fc2bb_in[:])
        ident = const.tile([P, P], F32)
        make_identity(nc, ident[:])
        ident_r = const.tile([P, P], FDT, tag="ident_r")
        nc.vector.tensor_copy(out=ident_r[:], in_=ident[:])

        # ---- question path: qq = relu(qe@fc0+fc0_b)@fc1b + fc1_b
        qe_sb = const.tile([P, 6 * G], F32)
        fc0w_sb = const.tile([P, 6 * P], F32)
        for k in range(6):
            nc.sync.dma_start(
                out=qe_sb[:, k * G : (k + 1) * G], in_=qeT_in[k * P : (k + 1) * P, :]
            )
            nc.sync.dma_start(
                out=fc0w_sb[:, k * P : (k + 1) * P],
                in_=fc0w_in[k * P : (k + 1) * P, :],
            )
        fc0bb_sb = const.tile([P, P], F32)
        nc.sync.dma_start(out=fc0bb_sb[:], in_=fc0bb_in[:])
        fc1b_sb = const.tile([P, P], F32)
        nc.sync.dma_start(out=fc1b_sb[:], in_=fc1b_in[:])
        fc1bb_sb = const.tile([P, P], F32)
        nc.sync.dma_start(out=fc1bb_sb[:], in_=fc1bb_in[:])

        pq = psp.tile([G, P], F32, space="PSUM", tag="mm")
        for k in range(6):
            nc.tensor.matmul(
                out=pq[:],
                lhsT=qe_sb[:, k * G : (k + 1) * G],
                rhs=fc0w_sb[:, k * P : (k + 1) * P],
                start=(k == 0),
                stop=(k == 5),
            )
        qtmp = epi.tile([G, P], F32, tag="qtmp")
        nc.vector.tensor_tensor(
            out=qtmp[:], in0=pq[:], in1=fc0bb_sb[:G, :], op=mybir.AluOpType.add
        )
        qrelu = epi.tile([G, P], F32, tag="qrelu")
        nc.scalar.activation(
            out=qrelu[:], in_=qtmp[:], func=mybir.ActivationFunctionType.Relu
        )
        pqt = pst.tile([P, G], F32, space="PSUM", tag="pt")
        nc.tensor.transpose(out=pqt[:], in_=qrelu[:], identity=ident[:G, :G])
        qT = epi.tile([P, G], F32, tag="qT")
        nc.scalar.copy(out=qT[:], in_=pqt[:])
        pqq = psp.tile([G, P], F32, space="PSUM", tag="mm")
        nc.tensor.matmul(
            out=pqq[:], lhsT=qT[:], rhs=fc1b_sb[:], start=True, stop=True
        )
        qq_sb = epi.tile([G, P], F32, tag="qq_sb")
        nc.vector.tensor_tensor(
            out=qq_sb[:], in0=pqq[:], in1=fc1bb_sb[:G, :], op=mybir.AluOpType.add
        )
        nc.sync.dma_start(out=qq_d[:], in_=qq_sb[:])

        # resident own-slice h~ buffers (self-loop term source), layer parity
        hs_keep = [
            const.tile([P, SLOT_ROWS], FDT, tag=f"hsk{i}", name=f"hsk{i}")
            for i in range(2)
        ]

        def allgather_block(l, j):
            r0 = j * SPB * P
            r1 = (j + 1) * SPB * P
            nc.gpsimd.collective_compute(
                "AllGather",
                mybir.AluOpType.bypass,
                replica_groups=[list(range(NCORES))],
                ins=[cc_in[r0:r1].opt()],
                outs=[tables[l][j * NCORES * SPB * P : (j + 1) * NCORES * SPB * P].opt()],
            )

        # ---- layer 0 production: h~0 = dinv * (x @ W0)
        for s in range(TPC):
            pp = psp.tile([P, P], F32, space="PSUM", tag="mm")
            nc.tensor.matmul(
                out=pp[:],
                lhsT=xT_sb[:, s * P : (s + 1) * P],
                rhs=W_sb[0][:],
                start=True,
                stop=True,
            )
            hs = hs_keep[0][:, s * P : (s + 1) * P]
            nc.scalar.activation(
                out=hs,
                in_=pp[:],
                func=mybir.ActivationFunctionType.Copy,
                scale=dinv_sb[:, s : s + 1],
            )
            nc.sync.dma_start(out=cc_in[s * P : (s + 1) * P, :], in_=hs)


        # whole-table AllGather for layer 0 (rank-major layout)
        nc.gpsimd.collective_compute(
            "AllGather",
            mybir.AluOpType.bypass,
            replica_groups=[list(range(NCORES))],
            ins=[cc_in[:].opt()],
            outs=[tables[0][:].opt()],
        )

        # ---- prefetch all qq rows (by graph id) into SBUF while AG0 runs
        qg_all = const.tile([P, TPC * P], F32)
        for s in range(TPC):
            nc.gpsimd.indirect_dma_start(
                out=qg_all[:, s * P : (s + 1) * P],
                out_offset=None,
                in_=qq_d[:],
                in_offset=bass.IndirectOffsetOnAxis(ap=gid_sb[:, s : s + 1], axis=0),
            )

        # ---- 3 aggregation layers
        for l in range(3):
            table = tables[l]
            for s in range(TPC):
                nch = int(chunks[s])
                cb = int(chunk_base[s])
                ps = psagg.tile([P, P], F32, space="PSUM", tag="agg")
                for k in range(nch):
                    g = gp.tile([P, P], FDT, tag="g")
                    nc.gpsimd.indirect_dma_start(
                        out=g[:],
                        out_offset=None,
                        in_=table[:],
                        in_offset=bass.IndirectOffsetOnAxis(
                            ap=(idx0_sb if l == 0 else idxB_sb)[
                                :, cb + k : cb + k + 1
                            ],
                            axis=0,
                        ),
                    )
                    oh = ohp.tile([P, P], FDT, tag="oh")
                    nc.vector.tensor_tensor(
                        out=oh[:],
                        in0=dstin_sb[:, cb + k : cb + k + 1].to_broadcast([P, P]),
                        in1=iota_sb[:],
                        op=mybir.AluOpType.is_equal,
                    )
                    nc.tensor.matmul(
                        out=ps[:],
                        lhsT=oh[:],
                        rhs=g[:],
                        start=(k == 0),
                        stop=False,
                    )
                # self-loop term: ps += I @ hs_keep[l%2][:, slot]
                nc.tensor.matmul(
                    out=ps[:],
                    lhsT=ident_r[:],
                    rhs=hs_keep[l % 2][:, s * P : (s + 1) * P],
                    start=False,
                    stop=True,
                )
                # epilogue: h = relu(dinv*agg + b)
                t1 = epi.tile([P, P], F32, tag="t1")
                nc.scalar.activation(
                    out=t1[:],
                    in_=ps[:],
                    func=mybir.ActivationFunctionType.Copy,
                    scale=dinv_sb[:, s : s + 1],
                )
                t2 = epi.tile([P, P], F32, tag="t2")
                nc.vector.tensor_tensor(
                    out=t2[:], in0=t1[:], in1=bb_sb[l][:], op=mybir.AluOpType.add
                )
                hrelu = epi.tile([P, P], F32, tag="hrelu")
                nc.scalar.activation(
                    out=hrelu[:], in_=t2[:], func=mybir.ActivationFunctionType.Relu
                )
                pt = pst.tile([P, P], F32, space="PSUM", tag="pt")
                nc.tensor.transpose(out=pt[:], in_=hrelu[:], identity=ident[:])
                hT = epi.tile([P, P], F32, tag="hT")
                nc.scalar.copy(out=hT[:], in_=pt[:])
                if l < 2:
                    # produce next layer h~ and stage for allgather
                    pp2 = psp.tile([P, P], F32, space="PSUM", tag="mm")
                    nc.tensor.matmul(
                        out=pp2[:], lhsT=hT[:], rhs=W_sb[l + 1][:], start=True, stop=True
                    )
                    hs2 = hs_keep[(l + 1) % 2][:, s * P : (s + 1) * P]
                    nc.scalar.activation(
                        out=hs2,
                        in_=pp2[:],
                        func=mybir.ActivationFunctionType.Copy,
                        scale=dinv_sb[:, s : s + 1],
                    )
                    nc.sync.dma_start(out=cc_in[s * P : (s + 1) * P, :], in_=hs2)
                    if (s + 1) % SPB == 0:
                        allgather_block(l + 1, s // SPB)
                else:
                    # MLP head: out = relu(h3@fc1a + qq[gid]) @ fc2 + fc2_b
                    pm = psp.tile([P, P], F32, space="PSUM", tag="mm")
                    nc.tensor.matmul(
                        out=pm[:], lhsT=hT[:], rhs=fc1a_sb[:], start=True, stop=True
                    )
                    u = epi.tile([P, P], F32, tag="u")
                    nc.vector.tensor_tensor(
                        out=u[:],
                        in0=pm[:],
                        in1=qg_all[:, s * P : (s + 1) * P],
                        op=mybir.AluOpType.add,
                    )
                    ur = epi.tile([P, P], F32, tag="ur")
                    nc.scalar.activation(
                        out=ur[:], in_=u[:], func=mybir.ActivationFunctionType.Relu
                    )
                    pt2 = pst.tile([P, P], F32, space="PSUM", tag="pt")
                    nc.tensor.transpose(out=pt2[:], in_=ur[:], identity=ident[:])
                    uT = epi.tile([P, P], F32, tag="uT")
                    nc.scalar.copy(out=uT[:], in_=pt2[:])
                    po = psp.tile([P, OUTC], F32, space="PSUM", tag="mm")
                    nc.tensor.matmul(
                        out=po[:], lhsT=uT[:], rhs=fc2w_sb[:], start=True, stop=True
                    )
                    ob = epi.tile([P, OUTC], F32, tag="ob")
                    nc.vector.tensor_tensor(
                        out=ob[:], in0=po[:], in1=fc2bb_sb[:], op=mybir.AluOpType.add
                    )
                    nc.sync.dma_start(out=out_d[s * P : (s + 1) * P, :], in_=ob[:])
    nc.compile()
    return nc


# ---------------------------------------------------------------- interface
_CACHE = {}


def kernel(**inputs):
    trace = bool(int(os.environ.get("GCN_TRACE", "0")))
    if trace:
        _install_axon_prof()
    from concourse.bass_utils import run_bass_kernel_spmd

    x = np.ascontiguousarray(np.asarray(inputs["x"], dtype=np.float32))
    qe = np.asarray(inputs["question_embedding"], dtype=np.float32)
    pp = preprocess(inputs["edge_index"], inputs["batch"])
    chunks = pp["chunks"]

    key = tuple(chunks.tolist())
    if key not in _CACHE:
        _CACHE[key] = build_program(chunks)
    nc = _CACHE[key]

    fdt = np.dtype("bfloat16") if BF16 else np.float32
    W = [np.asarray(inputs[f"W{i}"], np.float32) for i in range(3)]
    b = [np.asarray(inputs[f"b{i}"], np.float32) for i in range(3)]
    fc0_w = np.asarray(inputs["fc0_w"], np.float32)
    fc0_b = np.asarray(inputs["fc0_b"], np.float32)
    fc1_w = np.asarray(inputs["fc1_w"], np.float32)
    fc1_b = np.asarray(inputs["fc1_b"], np.float32)
    fc2_w = np.asarray(inputs["fc2_w"], np.float32)
    fc2_b = np.asarray(inputs["fc2_b"], np.float32)

    iota = np.broadcast_to(np.arange(P, dtype=np.float32), (P, P)).astype(fdt)
    common = {
        "iota_in": np.ascontiguousarray(iota),
        "W0": W[0],
        "W1": W[1],
        "W2": W[2],
        "bb0": np.broadcast_to(b[0], (P, P)).copy(),
        "bb1": np.broadcast_to(b[1], (P, P)).copy(),
        "bb2": np.broadcast_to(b[2], (P, P)).copy(),
        "qeT": np.ascontiguousarray(qe.T),
        "fc0w": fc0_w,
        "fc0bb": np.broadcast_to(fc0_b, (P, P)).copy(),
        "fc1a": np.ascontiguousarray(fc1_w[:P]),
        "fc1b": np.ascontiguousarray(fc1_w[P:]),
        "fc1bb": np.broadcast_to(fc1_b, (P, P)).copy(),
        "fc2w": fc2_w,
        "fc2bb": np.broadcast_to(fc2_b, (P, OUTC)).copy(),
    }

    in_maps = []
    for c in range(NCORES):
        xTc = np.zeros((P, SLOT_ROWS), dtype=np.float32)
        valid = pp["node_perm"][c] >= 0
        xTc[:, valid] = x[pp["node_perm"][c][valid]].T
        m = dict(common)
        m["xT"] = xTc
        m["idx0"] = np.ascontiguousarray(pp["src_T0"][c])
        m["idxB"] = np.ascontiguousarray(pp["src_TB"][c])
        m["dstin"] = np.ascontiguousarray(pp["dstin_T"][c].astype(fdt))
        m["dinv_in"] = np.ascontiguousarray(pp["dinv_slot"][c])
        m["gid_in"] = np.ascontiguousarray(pp["gid_slot"][c])
        in_maps.append(m)

    res = run_bass_kernel_spmd(
        nc,
        in_maps,
        list(range(NCORES)),
        trace=trace,
    )
    kernel.last_result = res

    out = np.zeros((N, OUTC), dtype=np.float32)
    for c in range(NCORES):
        valid = pp["node_perm"][c] >= 0
        out[pp["node_perm"][c][valid]] = res.results[c]["out"][valid]
    return out



# revision 11
# speedup vs baseline: 1.3135x; 1.3135x over previous
"""Trainium2 Bass kernel for the GCN model (nn_GCNModel_57853209477141).

Model: 3x GCNConv(128->128, sym-norm with self loops) with ReLU, question
embedding MLP, concat, 2-layer MLP head -> [50000, 32].

Strategy (8 NeuronCores, single SPMD launch):
- dst-node sharding: global tiles of 128 nodes; snake-dealt across cores by
  edge count so one compile-time chunk schedule serves all 8 cores.
- GCN norm factorization: agg[v] = dinv[v] * sum_{e->v} (dinv*h)[src_e]; the
  per-edge norm disappears by storing h~ = dinv*h in the gather table.
- table layout is block-major for ALL layers (AllGather fired per 7-slot
  block from the epilogues, overlapping collective traffic with compute).
- aggregation per slot: big bf16 dma_gather calls (one per slot-group x
  table-half; int16 gather indices address at most 32768 rows, so the table
  is split at row 32768) + batched one-hot build on DVE + matmul-accumulate
  into PSUM. Self-loop comes from a resident h~ slice via identity matmul;
  the bias enters as a rank-1 matmul (1/dinv outer b) so the epilogue is a
  single fused Relu(scale=dinv) activation.
- question path: qq = relu(qe@fc0+b)@fc1[128:] + fc1_b computed once on
  device, then one dma_gather by graph id;
  out = relu(h3@fc1[:128] + qq_g) @ fc2 + fc2_b.

Host preprocessing is index work only (sharding, edge sort, index planes);
all O(E*F) / O(N*F*F) float work runs on device.
"""
import os
import sys
import types
from contextlib import ExitStack

import numpy as np

# ---------------------------------------------------------------- constants
N = 50000
E = 800000
G = 64
P = 128
NCORES = 8
TPC = 49  # tile slots per core
SPB = 7  # slots per AllGather block
NBLK = 7
SLOT_ROWS = TPC * P  # 6272
NT = NCORES * SLOT_ROWS  # 50176
HALF = 32768  # int16 gather index limit -> table split row
QD = 768
OUTC = 32
GROUP = 2  # slots per dma_gather group

BF16 = np.dtype("bfloat16")


def _install_axon_prof():
    """Register NTFF profile hook if the image's antenv lacks it; neuter
    bucket upload (zero-egress). Harmless when running without tracing."""
    try:
        from antenv import axon_hooks  # noqa: F401
    except ImportError:
        try:
            import antenv
            from trn_agent_boot.trn_boot import _ntff_profile_via_ctypes

            hook = _ntff_profile_via_ctypes("/opt/axon/libaxon_pjrt.so")
            mod = types.ModuleType("antenv.axon_hooks")
            mod.get_axon_ntff_profile_hook = lambda: hook
            mod.set_axon_ntff_profile_hook = lambda h: None
            sys.modules["antenv.axon_hooks"] = mod
            antenv.axon_hooks = mod
        except Exception:
            pass
    try:
        import concourse.bass_utils as bu

        bu.upload_artifacts = lambda tmpdir: "local://" + str(tmpdir)
    except Exception:
        pass


def _wrap16(arr):
    """int array -> [128, len/16] int16 plane (idx i at partition i%16,
    col i//16; replicated to all 8 gpsimd core groups)."""
    m = np.asarray(arr, dtype=np.int16).reshape(-1, 16).T
    return np.tile(m, (8, 1))


# ---------------------------------------------------------------- host prep
def preprocess(edge_index, batch):
    src = np.asarray(edge_index[0], dtype=np.int64)
    dst = np.asarray(edge_index[1], dtype=np.int64)
    deg = (np.bincount(dst, minlength=N) + 1).astype(np.float64)
    dinv = (1.0 / np.sqrt(deg)).astype(np.float32)
    sqdeg = np.sqrt(deg).astype(np.float32)

    n_tiles = (N + P - 1) // P  # 391
    tile_of_node = np.arange(N) // P
    dst_tile = dst // P
    tile_counts = np.bincount(dst_tile, minlength=n_tiles)

    # snake-deal tiles (sorted by edge count desc) across cores
    order_all = np.argsort(-tile_counts, kind="stable")
    core_tiles = [[] for _ in range(NCORES)]
    for r in range(TPC):
        batch_t = order_all[r * NCORES : (r + 1) * NCORES]
        seq = range(NCORES) if r % 2 == 0 else range(NCORES - 1, -1, -1)
        for j, c in enumerate(seq):
            core_tiles[c].append(int(batch_t[j]) if j < len(batch_t) else -1)

    core_of_tile = np.full(n_tiles, -1, dtype=np.int64)
    slot_of_tile = np.full(n_tiles, -1, dtype=np.int64)
    for c in range(NCORES):
        for s, t in enumerate(core_tiles[c]):
            if t >= 0:
                core_of_tile[t] = c
                slot_of_tile[t] = s

    # block-major table row for every node (same layout for all 3 layers)
    blk = slot_of_tile[tile_of_node] // SPB
    table_row = (
        blk * (NCORES * SPB * P)
        + core_of_tile[tile_of_node] * (SPB * P)
        + (slot_of_tile[tile_of_node] % SPB) * P
        + (np.arange(N) % P)
    )

    order = np.argsort(dst_tile, kind="stable")
    src_sorted = src[order]
    dst_sorted = dst[order]
    sorted_tiles = dst_tile[order]
    tile_starts = np.searchsorted(sorted_tiles, np.arange(n_tiles))
    tile_ends = np.searchsorted(sorted_tiles, np.arange(n_tiles), side="right")

    src_rows = table_row[src_sorted]
    is_lo = src_rows < HALF

    # per-(core, slot, half) edge lists + common chunk schedule
    cnt = np.zeros((NCORES, TPC, 2), dtype=np.int64)
    elists = [[None] * TPC for _ in range(NCORES)]  # (rows_lo, din_lo, rows_hi, din_hi)
    for c in range(NCORES):
        for s in range(TPC):
            t = core_tiles[c][s]
            if t < 0:
                elists[c][s] = (
                    np.zeros(0, np.int64), np.zeros(0, np.int64),
                    np.zeros(0, np.int64), np.zeros(0, np.int64),
                )
                continue
            lo_, hi_ = tile_starts[t], tile_ends[t]
            rows = src_rows[lo_:hi_]
            din = dst_sorted[lo_:hi_] % P
            m = is_lo[lo_:hi_]
            elists[c][s] = (rows[m], din[m], rows[~m] - HALF, din[~m])
            cnt[c, s, 0] = int(m.sum())
            cnt[c, s, 1] = int((~m).sum())

    nch = np.ceil(cnt.max(axis=0) / P).astype(np.int64)  # [TPC, 2]

    # group schedule: chunk columns [grp lo (slot-major) | grp hi (slot-major)]
    groups = [list(range(g, min(g + GROUP, TPC))) for g in range(0, TPC, GROUP)]
    slot_base = np.zeros((TPC, 2), dtype=np.int64)
    grp_info = []  # (slots, col0, lo_tot, hi_tot)
    cur = 0
    for gs in groups:
        col0 = cur
        for s in gs:
            slot_base[s, 0] = cur
            cur += nch[s, 0]
        for s in gs:
            slot_base[s, 1] = cur
            cur += nch[s, 1]
        lo_tot = int(nch[gs, 0].sum())
        hi_tot = int(nch[gs, 1].sum())
        grp_info.append((gs, col0, lo_tot, hi_tot))
    TCH = int(cur)

    # per-core planes
    idx_T = np.zeros((NCORES, 128, TCH * 8), dtype=np.int16)
    dstin_T = np.full((NCORES, 128, TCH), -1.0, dtype=np.float32)
    for c in range(NCORES):
        for s in range(TPC):
            rows_lo, din_lo, rows_hi, din_hi = elists[c][s]
            for h, (rows, din) in enumerate(((rows_lo, din_lo), (rows_hi, din_hi))):
                nchunks = int(nch[s, h])
                if nchunks == 0:
                    continue
                b = int(slot_base[s, h])
                pad = nchunks * P
                rbuf = np.zeros(pad, dtype=np.int64)
                rbuf[: len(rows)] = rows
                dbuf = np.full(pad, -1.0, dtype=np.float32)
                dbuf[: len(din)] = din.astype(np.float32)
                idx_T[c, :, b * 8 : (b + nchunks) * 8] = _wrap16(rbuf)
                dstin_T[c, :, b : b + nchunks] = dbuf.reshape(nchunks, P).T

    DII_BLKS = (TPC + 2) // 3
    dinv_slot = np.zeros((NCORES, P, TPC), dtype=np.float32)
    # slot s -> partition (s%3)*32, column block s//3 (PE base-partition rule)
    dinvinv_slot = np.zeros((NCORES, P, DII_BLKS * P), dtype=np.float32)
    gid_lin = np.zeros((NCORES, SLOT_ROWS), dtype=np.int64)
    node_perm = np.full((NCORES, SLOT_ROWS), -1, dtype=np.int64)
    batch = np.asarray(batch, dtype=np.int64)
    for c in range(NCORES):
        for s in range(TPC):
            t = core_tiles[c][s]
            if t < 0:
                continue
            v0 = t * P
            v1 = min(v0 + P, N)
            n = v1 - v0
            dinv_slot[c, :n, s] = dinv[v0:v1]
            dinvinv_slot[c, (s % 3) * 32, (s // 3) * P : (s // 3) * P + n] = sqdeg[v0:v1]
            gid_lin[c, s * P : s * P + n] = batch[v0:v1]
            node_perm[c, s * P : s * P + n] = np.arange(v0, v1)

    gidq_T = np.stack([_wrap16(gid_lin[c]) for c in range(NCORES)])

    return dict(
        nch=nch,
        grp_info=grp_info,
        slot_base=slot_base,
        TCH=TCH,
        idx_T=idx_T,
        dstin_T=dstin_T,
        dinv_slot=dinv_slot,
        dinvinv_slot=dinvinv_slot,
        gidq_T=gidq_T,
        node_perm=node_perm,
    )


# ------------------------------------------------------------- bass program
def build_program(nch, grp_info, slot_base, TCH):
    import concourse.bacc as bacc
    import concourse.bass as bass
    import concourse.tile as tile
    from concourse import library_config, mybir
    from concourse.masks import make_identity

    FDT = mybir.dt.bfloat16
    F32 = mybir.dt.float32
    I16 = mybir.dt.int16

    NCHG = max(lo + hi for _, _, lo, hi in grp_info)  # chunks per group
    GHMAX = max(max(lo, hi) for _, _, lo, hi in grp_info)

    nc = bacc.Bacc("TRN2", target_bir_lowering=False)
    dp = nc.declare_dram_parameter
    xT = dp("xT", [P, SLOT_ROWS], F32, isOutput=False)
    idx_in = dp("idx_in", [P, TCH * 8], I16, isOutput=False)
    dstin = dp("dstin", [P, TCH], FDT, isOutput=False)
    iota_in = dp("iota_in", [P, NCHG * P], FDT, isOutput=False)
    DII_BLKS = (TPC + 2) // 3
    dinv_in = dp("dinv_in", [P, TPC], F32, isOutput=False)
    dinvinv_in = dp("dinvinv_in", [P, DII_BLKS * P], FDT, isOutput=False)
    gidq_in = dp("gidq_in", [P, TPC * 8], I16, isOutput=False)
    W_in = [dp(f"W{i}", [P, P], F32, isOutput=False) for i in range(3)]
    brow_in = dp("brow", [P, 3 * P], FDT, isOutput=False)
    qeT_in = dp("qeT", [QD, G], F32, isOutput=False)
    fc0w_in = dp("fc0w", [QD, P], F32, isOutput=False)
    fc0bb_in = dp("fc0bb", [P, P], F32, isOutput=False)
    fc1a_in = dp("fc1a", [P, P], F32, isOutput=False)
    fc1b_in = dp("fc1b", [P, P], F32, isOutput=False)
    fc1bb_in = dp("fc1bb", [P, P], F32, isOutput=False)
    fc2w_in = dp("fc2w", [P, OUTC], F32, isOutput=False)
    fc2bb_in = dp("fc2bb", [P, OUTC], F32, isOutput=False)
    ones_in = dp("ones_in", [1, P], F32, isOutput=False)
    out_d = dp("out", [SLOT_ROWS, OUTC], F32, isOutput=True)

    cc_in = nc.dram_tensor("cc_in", [SLOT_ROWS, P], FDT)
    tables = [
        nc.dram_tensor(f"table{l}", [NT, P], FDT, addr_space="Shared")
        for l in range(3)
    ]
    qq_d = nc.dram_tensor("qq_d", [G, P], FDT)

    with tile.TileContext(nc) as tc, ExitStack() as ctx:
        nc.gpsimd.load_library(library_config.mlp)

        const = ctx.enter_context(tc.tile_pool(name="const", bufs=1))
        xp = ctx.enter_context(tc.tile_pool(name="xp", bufs=2))
        gp = ctx.enter_context(tc.tile_pool(name="gp", bufs=4))
        ohp = ctx.enter_context(tc.tile_pool(name="ohp", bufs=3))
        psagg = ctx.enter_context(tc.tile_pool(name="psagg", bufs=3, space="PSUM"))
        psp = ctx.enter_context(tc.tile_pool(name="psp", bufs=3, space="PSUM"))
        pst = ctx.enter_context(tc.tile_pool(name="pst", bufs=2, space="PSUM"))
        epi = ctx.enter_context(tc.tile_pool(name="epi", bufs=4))

        # ---- constants
        idx_sb = const.tile([P, TCH * 8], I16)
        nc.scalar.dma_start(out=idx_sb[:], in_=idx_in[:])
        dstin_sb = const.tile([P, TCH], FDT)
        nc.scalar.dma_start(out=dstin_sb[:], in_=dstin[:])
        iota_sb = const.tile([P, NCHG * P], FDT)
        nc.scalar.dma_start(out=iota_sb[:], in_=iota_in[:])
        dinv_sb = const.tile([P, TPC], F32)
        nc.sync.dma_start(out=dinv_sb[:], in_=dinv_in[:])
        dinvinv_sb = const.tile([P, DII_BLKS * P], FDT)
        nc.sync.dma_start(out=dinvinv_sb[:], in_=dinvinv_in[:])
        gidq_sb = const.tile([P, TPC * 8], I16)
        nc.sync.dma_start(out=gidq_sb[:], in_=gidq_in[:])
        W_sb = []
        for i in range(3):
            w = const.tile([P, P], F32, tag=f"W{i}")
            nc.sync.dma_start(out=w[:], in_=W_in[i][:])
            W_sb.append(w)
        brow_sb = const.tile([P, 3 * P], FDT)
        nc.sync.dma_start(out=brow_sb[:], in_=brow_in[:])
        fc1a_sb = const.tile([P, P], F32)
        nc.sync.dma_start(out=fc1a_sb[:], in_=fc1a_in[:])
        fc2w_sb = const.tile([P, OUTC], F32)
        nc.sync.dma_start(out=fc2w_sb[:], in_=fc2w_in[:])
        fc2bb_sb = const.tile([P, OUTC], F32)
        nc.sync.dma_start(out=fc2bb_sb[:], in_=fc2bb_in[:])
        ones_sb = const.tile([1, P], F32)
        nc.sync.dma_start(out=ones_sb[:], in_=ones_in[:])
        ident = const.tile([P, P], F32)
        make_identity(nc, ident[:])
        ident_r = const.tile([P, P], FDT, tag="ident_r")
        nc.vector.tensor_copy(out=ident_r[:], in_=ident[:])

        # ---- question path: qq = relu(qe@fc0+fc0_b)@fc1b + fc1_b  (bf16 out)
        qe_sb = const.tile([P, 6 * G], F32)
        fc0w_sb = const.tile([P, 6 * P], F32)
        for k in range(6):
            nc.sync.dma_start(
                out=qe_sb[:, k * G : (k + 1) * G], in_=qeT_in[k * P : (k + 1) * P, :]
            )
            nc.sync.dma_start(
                out=fc0w_sb[:, k * P : (k + 1) * P],
                in_=fc0w_in[k * P : (k + 1) * P, :],
            )
        fc0bb_sb = const.tile([P, P], F32)
        nc.sync.dma_start(out=fc0bb_sb[:], in_=fc0bb_in[:])
        fc1b_sb = const.tile([P, P], F32)
        nc.sync.dma_start(out=fc1b_sb[:], in_=fc1b_in[:])
        fc1bb_sb = const.tile([P, P], F32)
        nc.sync.dma_start(out=fc1bb_sb[:], in_=fc1bb_in[:])

        pq = psp.tile([G, P], F32, space="PSUM", tag="mm")
        for k in range(6):
            nc.tensor.matmul(
                out=pq[:],
                lhsT=qe_sb[:, k * G : (k + 1) * G],
                rhs=fc0w_sb[:, k * P : (k + 1) * P],
                start=(k == 0),
                stop=(k == 5),
            )
        qtmp = epi.tile([G, P], F32, tag="qtmp")
        nc.vector.tensor_tensor(
            out=qtmp[:], in0=pq[:], in1=fc0bb_sb[:G, :], op=mybir.AluOpType.add
        )
        qrelu = epi.tile([G, P], F32, tag="qrelu")
        nc.scalar.activation(
            out=qrelu[:], in_=qtmp[:], func=mybir.ActivationFunctionType.Relu
        )
        pqt = pst.tile([P, G], F32, space="PSUM", tag="pt")
        nc.tensor.transpose(out=pqt[:], in_=qrelu[:], identity=ident[:G, :G])
        qT = epi.tile([P, G], F32, tag="qT")
        nc.scalar.copy(out=qT[:], in_=pqt[:])
        pqq = psp.tile([G, P], F32, space="PSUM", tag="mm")
        nc.tensor.matmul(
            out=pqq[:], lhsT=qT[:], rhs=fc1b_sb[:], start=True, stop=True
        )
        qq_sb = epi.tile([G, P], FDT, tag="qq_sb")
        nc.vector.tensor_tensor(
            out=qq_sb[:], in0=pqq[:], in1=fc1bb_sb[:G, :], op=mybir.AluOpType.add
        )
        nc.sync.dma_start(out=qq_d[:], in_=qq_sb[:])

        # resident own-slice h~ buffers (self-loop term source), layer parity
        hs_keep = [
            const.tile([P, SLOT_ROWS], FDT, tag=f"hsk{i}", name=f"hsk{i}")
            for i in range(2)
        ]

        def allgather_block(l, j):
            r0 = j * SPB * P
            r1 = (j + 1) * SPB * P
            nc.gpsimd.collective_compute(
                "AllGather",
                mybir.AluOpType.bypass,
                replica_groups=[list(range(NCORES))],
                ins=[cc_in[r0:r1].opt()],
                outs=[tables[l][j * NCORES * SPB * P : (j + 1) * NCORES * SPB * P].opt()],
            )

        # ---- layer 0 production: h~0 = dinv * (x @ W0), per AG block
        for j in range(NBLK):
            xb = xp.tile([P, SPB * P], F32, tag="xb")
            nc.sync.dma_start(out=xb[:], in_=xT[:, j * SPB * P : (j + 1) * SPB * P])
            for i in range(SPB):
                s = j * SPB + i
                pp = psp.tile([P, P], F32, space="PSUM", tag="mm")
                nc.tensor.matmul(
                    out=pp[:],
                    lhsT=xb[:, i * P : (i + 1) * P],
                    rhs=W_sb[0][:],
                    start=True,
                    stop=True,
                )
                hs = hs_keep[0][:, s * P : (s + 1) * P]
                nc.scalar.activation(
                    out=hs,
                    in_=pp[:],
                    func=mybir.ActivationFunctionType.Copy,
                    scale=dinv_sb[:, s : s + 1],
                )
                nc.sync.dma_start(out=cc_in[s * P : (s + 1) * P, :], in_=hs)
            allgather_block(0, j)

        # ---- qq rows by graph id (single gather; overlaps with AG0)
        qg_all = const.tile([P, TPC * P], FDT)
        nc.gpsimd.dma_gather(
            out_ap=qg_all[:].rearrange("p (k q) -> p k q", q=P),
            in_ap=qq_d[:],
            idxs_ap=gidq_sb[:],
            num_idxs=TPC * P,
            num_idxs_reg=TPC * P,
            elem_size=P,
            single_packet=False,
        )

        # ---- 3 aggregation layers
        for l in range(3):
            table = tables[l]
            for gs, col0, lo_tot, hi_tot in grp_info:
                glo = gp.tile([P, GHMAX * P], FDT, tag="glo")
                if lo_tot:
                    nc.gpsimd.dma_gather(
                        out_ap=glo[:, : lo_tot * P].rearrange("p (k q) -> p k q", q=P),
                        in_ap=table[0:HALF],
                        idxs_ap=idx_sb[:, col0 * 8 : (col0 + lo_tot) * 8],
                        num_idxs=lo_tot * P,
                        num_idxs_reg=lo_tot * P,
                        elem_size=P,
                        single_packet=False,
                    )
                ghi = gp.tile([P, GHMAX * P], FDT, tag="ghi")
                if hi_tot:
                    nc.gpsimd.dma_gather(
                        out_ap=ghi[:, : hi_tot * P].rearrange("p (k q) -> p k q", q=P),
                        in_ap=table[HALF:NT],
                        idxs_ap=idx_sb[:, (col0 + lo_tot) * 8 : (col0 + lo_tot + hi_tot) * 8],
                        num_idxs=hi_tot * P,
                        num_idxs_reg=hi_tot * P,
                        elem_size=P,
                        single_packet=False,
                    )
                ntot = lo_tot + hi_tot
                oh = ohp.tile([P, NCHG * P], FDT, tag="oh")
                if ntot:
                    nc.vector.tensor_tensor(
                        out=oh[:, : ntot * P].rearrange("p (k q) -> p k q", q=P),
                        in0=dstin_sb[:, col0 : col0 + ntot].to_broadcast([P, ntot, P]),
                        in1=iota_sb[:, : ntot * P].rearrange("p (k q) -> p k q", q=P),
                        op=mybir.AluOpType.is_equal,
                    )
                for s in gs:
                    nlo = int(nch[s, 0])
                    nhi = int(nch[s, 1])
                    lo_off = int(slot_base[s, 0]) - col0
                    hi_off = int(slot_base[s, 1]) - col0
                    lo_rel = lo_off  # within glo
                    hi_rel = hi_off - lo_tot  # within ghi
                    ps = psagg.tile([P, P], F32, space="PSUM", tag="agg")
                    for k in range(nlo):
                        nc.tensor.matmul(
                            out=ps[:],
                            lhsT=oh[:, (lo_off + k) * P : (lo_off + k + 1) * P],
                            rhs=glo[:, (lo_rel + k) * P : (lo_rel + k + 1) * P],
                            start=(k == 0),
                            stop=False,
                        )
                    for k in range(nhi):
                        nc.tensor.matmul(
                            out=ps[:],
                            lhsT=oh[:, (hi_off + k) * P : (hi_off + k + 1) * P],
                            rhs=ghi[:, (hi_rel + k) * P : (hi_rel + k + 1) * P],
                            start=False,
                            stop=False,
                        )
                    # self-loop term
                    nc.tensor.matmul(
                        out=ps[:],
                        lhsT=ident_r[:],
                        rhs=hs_keep[l % 2][:, s * P : (s + 1) * P],
                        start=False,
                        stop=False,
                    )
                    # rank-1 bias: ps += (1/dinv) outer b  (so Relu(dinv*ps) is right)
                    bp = (s % 3) * 32
                    nc.tensor.matmul(
                        out=ps[:],
                        lhsT=dinvinv_sb[bp : bp + 1, (s // 3) * P : (s // 3 + 1) * P],
                        rhs=brow_sb[bp : bp + 1, l * P : (l + 1) * P],
                        start=False,
                        stop=True,
                    )
                    hrelu = epi.tile([P, P], F32, tag="hrelu")
                    nc.scalar.activation(
                        out=hrelu[:],
                        in_=ps[:],
                        func=mybir.ActivationFunctionType.Relu,
                        scale=dinv_sb[:, s : s + 1],
                    )
                    pt = pst.tile([P, P], F32, space="PSUM", tag="pt")
                    nc.tensor.transpose(out=pt[:], in_=hrelu[:], identity=ident[:])
                    hT = epi.tile([P, P], F32, tag="hT")
                    nc.vector.tensor_copy(out=hT[:], in_=pt[:])
                    if l < 2:
                        pp2 = psp.tile([P, P], F32, space="PSUM", tag="mm")
                        nc.tensor.matmul(
                            out=pp2[:], lhsT=hT[:], rhs=W_sb[l + 1][:],
                            start=True, stop=True,
                        )
                        hs2 = hs_keep[(l + 1) % 2][:, s * P : (s + 1) * P]
                        nc.scalar.activation(
                            out=hs2,
                            in_=pp2[:],
                            func=mybir.ActivationFunctionType.Copy,
                            scale=dinv_sb[:, s : s + 1],
                        )
                        nc.sync.dma_start(
                            out=cc_in[s * P : (s + 1) * P, :], in_=hs2
                        )
                        if (s + 1) % SPB == 0:
                            allgather_block(l + 1, s // SPB)
                    else:
                        # MLP head: out = relu(h3@fc1a + qq[gid]) @ fc2 + fc2_b
                        pm = psp.tile([P, P], F32, space="PSUM", tag="mm")
                        nc.tensor.matmul(
                            out=pm[:], lhsT=hT[:], rhs=fc1a_sb[:],
                            start=True, stop=True,
                        )
                        u = epi.tile([P, P], F32, tag="u")
                        nc.vector.tensor_tensor(
                            out=u[:],
                            in0=pm[:],
                            in1=qg_all[:, s * P : (s + 1) * P],
                            op=mybir.AluOpType.add,
                        )
                        ur = epi.tile([P, P], F32, tag="ur")
                        nc.scalar.activation(
                            out=ur[:], in_=u[:],
                            func=mybir.ActivationFunctionType.Relu,
                        )
                        pt2 = pst.tile([P, P], F32, space="PSUM", tag="pt")
                        nc.tensor.transpose(out=pt2[:], in_=ur[:], identity=ident[:])
                        uT = epi.tile([P, P], F32, tag="uT")
                        nc.vector.tensor_copy(out=uT[:], in_=pt2[:])
                        po = psp.tile([P, OUTC], F32, space="PSUM", tag="mm")
                        nc.tensor.matmul(
                            out=po[:], lhsT=uT[:], rhs=fc2w_sb[:],
                            start=True, stop=False,
                        )
                        nc.tensor.matmul(
                            out=po[:], lhsT=ones_sb[:], rhs=fc2bb_sb[0:1, :],
                            start=False, stop=True,
                        )
                        ob = epi.tile([P, OUTC], F32, tag="ob")
                        nc.vector.tensor_copy(out=ob[:], in_=po[:])
                        nc.sync.dma_start(
                            out=out_d[s * P : (s + 1) * P, :], in_=ob[:]
                        )
    nc.compile()
    return nc


# ---------------------------------------------------------------- interface
_CACHE = {}


def kernel(**inputs):
    trace = bool(int(os.environ.get("GCN_TRACE", "0")))
    if trace:
        _install_axon_prof()
    from concourse.bass_utils import run_bass_kernel_spmd

    x = np.ascontiguousarray(np.asarray(inputs["x"], dtype=np.float32))
    qe = np.asarray(inputs["question_embedding"], dtype=np.float32)
    pp = preprocess(inputs["edge_index"], inputs["batch"])
    nch = pp["nch"]

    key = tuple(nch.flatten().tolist())
    if key not in _CACHE:
        _CACHE[key] = build_program(nch, pp["grp_info"], pp["slot_base"], pp["TCH"])
    nc = _CACHE[key]

    NCHG = max(lo + hi for _, _, lo, hi in pp["grp_info"])

    W = [np.asarray(inputs[f"W{i}"], np.float32) for i in range(3)]
    b = [np.asarray(inputs[f"b{i}"], np.float32) for i in range(3)]
    fc0_w = np.asarray(inputs["fc0_w"], np.float32)
    fc0_b = np.asarray(inputs["fc0_b"], np.float32)
    fc1_w = np.asarray(inputs["fc1_w"], np.float32)
    fc1_b = np.asarray(inputs["fc1_b"], np.float32)
    fc2_w = np.asarray(inputs["fc2_w"], np.float32)
    fc2_b = np.asarray(inputs["fc2_b"], np.float32)

    iota = np.broadcast_to(np.arange(P, dtype=np.float32), (P, P))
    iota_rep = np.ascontiguousarray(np.tile(iota, (1, NCHG)).astype(BF16))
    common = {
        "iota_in": iota_rep,
        "W0": W[0],
        "W1": W[1],
        "W2": W[2],
        "brow": np.broadcast_to(np.concatenate(b), (P, 3 * P)).astype(BF16).copy(),
        "qeT": np.ascontiguousarray(qe.T),
        "fc0w": fc0_w,
        "fc0bb": np.broadcast_to(fc0_b, (P, P)).copy(),
        "fc1a": np.ascontiguousarray(fc1_w[:P]),
        "fc1b": np.ascontiguousarray(fc1_w[P:]),
        "fc1bb": np.broadcast_to(fc1_b, (P, P)).copy(),
        "fc2w": fc2_w,
        "fc2bb": np.broadcast_to(fc2_b, (P, OUTC)).copy(),
        "ones_in": np.ones((1, P), np.float32),
    }

    in_maps = []
    for c in range(NCORES):
        xTc = np.zeros((P, SLOT_ROWS), dtype=np.float32)
        valid = pp["node_perm"][c] >= 0
        xTc[:, valid] = x[pp["node_perm"][c][valid]].T
        m = dict(common)
        m["xT"] = xTc
        m["idx_in"] = np.ascontiguousarray(pp["idx_T"][c])
        m["dstin"] = np.ascontiguousarray(pp["dstin_T"][c].astype(BF16))
        m["dinv_in"] = np.ascontiguousarray(pp["dinv_slot"][c])
        m["dinvinv_in"] = np.ascontiguousarray(pp["dinvinv_slot"][c].astype(BF16))
        m["gidq_in"] = np.ascontiguousarray(pp["gidq_T"][c])
        in_maps.append(m)

    res = run_bass_kernel_spmd(
        nc,
        in_maps,
        list(range(NCORES)),
        trace=trace,
    )
    kernel.last_result = res

    out = np.zeros((N, OUTC), dtype=np.float32)
    for c in range(NCORES):
        valid = pp["node_perm"][c] >= 0
        out[pp["node_perm"][c][valid]] = res.results[c]["out"][valid]
    return out


# revision 14
# speedup vs baseline: 2.7381x; 2.0846x over previous
"""Trainium2 Bass kernel for the GCN model (nn_GCNModel_57853209477141).

Model: 3x GCNConv(128->128, sym-norm with self loops) with ReLU, question
embedding MLP, concat, 2-layer MLP head -> [50000, 32].

Strategy (8 NeuronCores, single SPMD launch):
- dst-node sharding: global tiles of 128 nodes; snake-dealt across cores by
  edge count so one compile-time chunk schedule serves all 8 cores.
- GCN norm factorization: agg[v] = dinv[v] * sum_{e->v} (dinv*h)[src_e]; the
  per-edge norm disappears by storing h~ = dinv*h in the gather table.
- table layout is block-major for ALL layers (AllGather fired per 7-slot
  block from the epilogues, overlapping collective traffic with compute).
- aggregation per slot: big bf16 dma_gather calls (one per slot-group x
  table-half; int16 gather indices address at most 32768 rows, so the table
  is split at row 32768) + batched one-hot build on DVE + matmul-accumulate
  into PSUM. Self-loop comes from a resident h~ slice via identity matmul;
  the bias enters as a rank-1 matmul (1/dinv outer b) so the epilogue is a
  single fused Relu(scale=dinv) activation.
- question path: qq = relu(qe@fc0+b)@fc1[128:] + fc1_b computed once on
  device, then one dma_gather by graph id;
  out = relu(h3@fc1[:128] + qq_g) @ fc2 + fc2_b.

Host preprocessing is index work only (sharding, edge sort, index planes);
all O(E*F) / O(N*F*F) float work runs on device.
"""
import os
import sys
import types
from contextlib import ExitStack

import numpy as np

# ---------------------------------------------------------------- constants
N = 50000
E = 800000
G = 64
P = 128
NCORES = 8
TPC = 49  # tile slots per core
SPB = 7  # slots per AllGather block
NBLK = 7
SLOT_ROWS = TPC * P  # 6272
NT = NCORES * SLOT_ROWS  # 50176
HALF = 32768  # int16 gather index limit -> table split row
QD = 768
OUTC = 32
GROUP = 2  # slots per dma_gather group

BF16 = np.dtype("bfloat16")


def _install_axon_prof():
    """Register NTFF profile hook if the image's antenv lacks it; neuter
    bucket upload (zero-egress). Harmless when running without tracing."""
    try:
        from antenv import axon_hooks  # noqa: F401
    except ImportError:
        try:
            import antenv
            from trn_agent_boot.trn_boot import _ntff_profile_via_ctypes

            hook = _ntff_profile_via_ctypes("/opt/axon/libaxon_pjrt.so")
            mod = types.ModuleType("antenv.axon_hooks")
            mod.get_axon_ntff_profile_hook = lambda: hook
            mod.set_axon_ntff_profile_hook = lambda h: None
            sys.modules["antenv.axon_hooks"] = mod
            antenv.axon_hooks = mod
        except Exception:
            pass
    try:
        import concourse.bass_utils as bu

        bu.upload_artifacts = lambda tmpdir: "local://" + str(tmpdir)
    except Exception:
        pass


def _wrap16(arr):
    """int array -> [128, len/16] int16 plane (idx i at partition i%16,
    col i//16; replicated to all 8 gpsimd core groups)."""
    m = np.asarray(arr, dtype=np.int16).reshape(-1, 16).T
    return np.tile(m, (8, 1))


# ---------------------------------------------------------------- host prep
def preprocess(edge_index, batch):
    src = np.asarray(edge_index[0], dtype=np.int64)
    dst = np.asarray(edge_index[1], dtype=np.int64)
    deg = (np.bincount(dst, minlength=N) + 1).astype(np.float64)
    dinv = (1.0 / np.sqrt(deg)).astype(np.float32)
    sqdeg = np.sqrt(deg).astype(np.float32)

    n_tiles = (N + P - 1) // P  # 391
    tile_of_node = np.arange(N) // P
    dst_tile = dst // P
    tile_counts = np.bincount(dst_tile, minlength=n_tiles)

    # snake-deal tiles (sorted by edge count desc) across cores
    order_all = np.argsort(-tile_counts, kind="stable")
    core_tiles = [[] for _ in range(NCORES)]
    for r in range(TPC):
        batch_t = order_all[r * NCORES : (r + 1) * NCORES]
        seq = range(NCORES) if r % 2 == 0 else range(NCORES - 1, -1, -1)
        for j, c in enumerate(seq):
            core_tiles[c].append(int(batch_t[j]) if j < len(batch_t) else -1)

    core_of_tile = np.full(n_tiles, -1, dtype=np.int64)
    slot_of_tile = np.full(n_tiles, -1, dtype=np.int64)
    for c in range(NCORES):
        for s, t in enumerate(core_tiles[c]):
            if t >= 0:
                core_of_tile[t] = c
                slot_of_tile[t] = s

    # block-major table row for every node (same layout for all 3 layers)
    blk = slot_of_tile[tile_of_node] // SPB
    table_row = (
        blk * (NCORES * SPB * P)
        + core_of_tile[tile_of_node] * (SPB * P)
        + (slot_of_tile[tile_of_node] % SPB) * P
        + (np.arange(N) % P)
    )

    order = np.argsort(dst_tile, kind="stable")
    src_sorted = src[order]
    dst_sorted = dst[order]
    sorted_tiles = dst_tile[order]
    tile_starts = np.searchsorted(sorted_tiles, np.arange(n_tiles))
    tile_ends = np.searchsorted(sorted_tiles, np.arange(n_tiles), side="right")

    src_rows = table_row[src_sorted]
    is_lo = src_rows < HALF

    # per-(core, slot, half) edge lists + common chunk schedule
    cnt = np.zeros((NCORES, TPC, 2), dtype=np.int64)
    elists = [[None] * TPC for _ in range(NCORES)]  # (rows_lo, din_lo, rows_hi, din_hi)
    for c in range(NCORES):
        for s in range(TPC):
            t = core_tiles[c][s]
            if t < 0:
                elists[c][s] = (
                    np.zeros(0, np.int64), np.zeros(0, np.int64),
                    np.zeros(0, np.int64), np.zeros(0, np.int64),
                )
                continue
            lo_, hi_ = tile_starts[t], tile_ends[t]
            rows = src_rows[lo_:hi_]
            din = dst_sorted[lo_:hi_] % P
            m = is_lo[lo_:hi_]
            elists[c][s] = (rows[m], din[m], rows[~m] - HALF, din[~m])
            cnt[c, s, 0] = int(m.sum())
            cnt[c, s, 1] = int((~m).sum())

    nch = np.ceil(cnt.max(axis=0) / P).astype(np.int64)  # [TPC, 2]

    # group schedule: chunk columns [grp lo (slot-major) | grp hi (slot-major)]
    groups = [list(range(g, min(g + GROUP, TPC))) for g in range(0, TPC, GROUP)]
    slot_base = np.zeros((TPC, 2), dtype=np.int64)
    grp_info = []  # (slots, col0, lo_tot, hi_tot)
    cur = 0
    for gs in groups:
        col0 = cur
        for s in gs:
            slot_base[s, 0] = cur
            cur += nch[s, 0]
        for s in gs:
            slot_base[s, 1] = cur
            cur += nch[s, 1]
        lo_tot = int(nch[gs, 0].sum())
        hi_tot = int(nch[gs, 1].sum())
        grp_info.append((gs, col0, lo_tot, hi_tot))
    TCH = int(cur)

    # per-core planes
    idx_T = np.zeros((NCORES, 128, TCH * 8), dtype=np.int16)
    dstin_T = np.full((NCORES, 128, TCH), -1.0, dtype=np.float32)
    for c in range(NCORES):
        for s in range(TPC):
            rows_lo, din_lo, rows_hi, din_hi = elists[c][s]
            for h, (rows, din) in enumerate(((rows_lo, din_lo), (rows_hi, din_hi))):
                nchunks = int(nch[s, h])
                if nchunks == 0:
                    continue
                b = int(slot_base[s, h])
                pad = nchunks * P
                rbuf = np.zeros(pad, dtype=np.int64)
                rbuf[: len(rows)] = rows
                dbuf = np.full(pad, -1.0, dtype=np.float32)
                dbuf[: len(din)] = din.astype(np.float32)
                idx_T[c, :, b * 8 : (b + nchunks) * 8] = _wrap16(rbuf)
                dstin_T[c, :, b : b + nchunks] = dbuf.reshape(nchunks, P).T

    DII_BLKS = (TPC + 2) // 3
    dinv_slot = np.zeros((NCORES, P, TPC), dtype=np.float32)
    # slot s -> partition (s%3)*32, column block s//3 (PE base-partition rule)
    dinvinv_slot = np.zeros((NCORES, P, DII_BLKS * P), dtype=np.float32)
    gid_lin = np.zeros((NCORES, SLOT_ROWS), dtype=np.int64)
    node_perm = np.full((NCORES, SLOT_ROWS), -1, dtype=np.int64)
    batch = np.asarray(batch, dtype=np.int64)
    for c in range(NCORES):
        for s in range(TPC):
            t = core_tiles[c][s]
            if t < 0:
                continue
            v0 = t * P
            v1 = min(v0 + P, N)
            n = v1 - v0
            dinv_slot[c, :n, s] = dinv[v0:v1]
            dinvinv_slot[c, (s % 3) * 32, (s // 3) * P : (s // 3) * P + n] = sqdeg[v0:v1]
            gid_lin[c, s * P : s * P + n] = batch[v0:v1]
            node_perm[c, s * P : s * P + n] = np.arange(v0, v1)

    gidq_T = np.stack([_wrap16(gid_lin[c]) for c in range(NCORES)])

    return dict(
        nch=nch,
        grp_info=grp_info,
        slot_base=slot_base,
        TCH=TCH,
        idx_T=idx_T,
        dstin_T=dstin_T,
        dinv_slot=dinv_slot,
        dinvinv_slot=dinvinv_slot,
        gidq_T=gidq_T,
        node_perm=node_perm,
    )


# ------------------------------------------------------------- bass program
def build_program(nch, grp_info, slot_base, TCH):
    import concourse.bacc as bacc
    import concourse.bass as bass
    import concourse.tile as tile
    from concourse import library_config, mybir
    from concourse.masks import make_identity

    FDT = mybir.dt.bfloat16
    F32 = mybir.dt.float32
    I16 = mybir.dt.int16

    NCHG = max(lo + hi for _, _, lo, hi in grp_info)  # chunks per group
    GHMAX = max(max(lo, hi) for _, _, lo, hi in grp_info)

    nc = bacc.Bacc("TRN2", target_bir_lowering=False, num_swdge_queues=4)
    dp = nc.declare_dram_parameter
    xT = dp("xT", [P, SLOT_ROWS], F32, isOutput=False)
    idx_in = dp("idx_in", [P, TCH * 8], I16, isOutput=False)
    dstin = dp("dstin", [P, TCH], FDT, isOutput=False)
    iota_in = dp("iota_in", [P, NCHG * P], FDT, isOutput=False)
    DII_BLKS = (TPC + 2) // 3
    dinv_in = dp("dinv_in", [P, TPC], F32, isOutput=False)
    dinvinv_in = dp("dinvinv_in", [P, DII_BLKS * P], FDT, isOutput=False)
    gidq_in = dp("gidq_in", [P, TPC * 8], I16, isOutput=False)
    W_in = [dp(f"W{i}", [P, P], F32, isOutput=False) for i in range(3)]
    brow_in = dp("brow", [P, 3 * P], FDT, isOutput=False)
    qeT_in = dp("qeT", [QD, G], F32, isOutput=False)
    fc0w_in = dp("fc0w", [QD, P], F32, isOutput=False)
    fc0bb_in = dp("fc0bb", [P, P], F32, isOutput=False)
    fc1a_in = dp("fc1a", [P, P], F32, isOutput=False)
    fc1b_in = dp("fc1b", [P, P], F32, isOutput=False)
    fc1bb_in = dp("fc1bb", [P, P], F32, isOutput=False)
    fc2w_in = dp("fc2w", [P, OUTC], F32, isOutput=False)
    fc2bb_in = dp("fc2bb", [P, OUTC], F32, isOutput=False)
    ones_in = dp("ones_in", [1, P], F32, isOutput=False)
    out_d = dp("out", [SLOT_ROWS, OUTC], F32, isOutput=True)

    cc_in = nc.dram_tensor("cc_in", [SLOT_ROWS, P], FDT)
    tables = [
        nc.dram_tensor(f"table{l}", [NT, P], FDT, addr_space="Shared")
        for l in range(3)
    ]
    qq_d = nc.dram_tensor("qq_d", [G, P], FDT)

    with tile.TileContext(nc) as tc, ExitStack() as ctx:
        nc.gpsimd.load_library(library_config.mlp)

        const = ctx.enter_context(tc.tile_pool(name="const", bufs=1))
        xp = ctx.enter_context(tc.tile_pool(name="xp", bufs=2))
        gp = ctx.enter_context(tc.tile_pool(name="gp", bufs=4))
        ohp = ctx.enter_context(tc.tile_pool(name="ohp", bufs=3))
        psagg = ctx.enter_context(tc.tile_pool(name="psagg", bufs=3, space="PSUM"))
        psp = ctx.enter_context(tc.tile_pool(name="psp", bufs=3, space="PSUM"))
        pst = ctx.enter_context(tc.tile_pool(name="pst", bufs=2, space="PSUM"))
        epi = ctx.enter_context(tc.tile_pool(name="epi", bufs=4))

        # ---- constants
        idx_sb = const.tile([P, TCH * 8], I16)
        nc.scalar.dma_start(out=idx_sb[:], in_=idx_in[:])
        dstin_sb = const.tile([P, TCH], FDT)
        nc.scalar.dma_start(out=dstin_sb[:], in_=dstin[:])
        iota_sb = const.tile([P, NCHG * P], FDT)
        nc.scalar.dma_start(out=iota_sb[:], in_=iota_in[:])
        dinv_sb = const.tile([P, TPC], F32)
        nc.sync.dma_start(out=dinv_sb[:], in_=dinv_in[:])
        dinvinv_sb = const.tile([P, DII_BLKS * P], FDT)
        nc.sync.dma_start(out=dinvinv_sb[:], in_=dinvinv_in[:])
        gidq_sb = const.tile([P, TPC * 8], I16)
        nc.sync.dma_start(out=gidq_sb[:], in_=gidq_in[:])
        W_sb = []
        for i in range(3):
            w = const.tile([P, P], F32, tag=f"W{i}")
            nc.sync.dma_start(out=w[:], in_=W_in[i][:])
            W_sb.append(w)
        brow_sb = const.tile([P, 3 * P], FDT)
        nc.sync.dma_start(out=brow_sb[:], in_=brow_in[:])
        fc1a_sb = const.tile([P, P], F32)
        nc.sync.dma_start(out=fc1a_sb[:], in_=fc1a_in[:])
        fc2w_sb = const.tile([P, OUTC], F32)
        nc.sync.dma_start(out=fc2w_sb[:], in_=fc2w_in[:])
        fc2bb_sb = const.tile([P, OUTC], F32)
        nc.sync.dma_start(out=fc2bb_sb[:], in_=fc2bb_in[:])
        ones_sb = const.tile([1, P], F32)
        nc.sync.dma_start(out=ones_sb[:], in_=ones_in[:])
        ident = const.tile([P, P], F32)
        make_identity(nc, ident[:])
        ident_r = const.tile([P, P], FDT, tag="ident_r")
        nc.vector.tensor_copy(out=ident_r[:], in_=ident[:])

        # ---- question path: qq = relu(qe@fc0+fc0_b)@fc1b + fc1_b  (bf16 out)
        qe_sb = const.tile([P, 6 * G], F32)
        fc0w_sb = const.tile([P, 6 * P], F32)
        for k in range(6):
            nc.sync.dma_start(
                out=qe_sb[:, k * G : (k + 1) * G], in_=qeT_in[k * P : (k + 1) * P, :]
            )
            nc.sync.dma_start(
                out=fc0w_sb[:, k * P : (k + 1) * P],
                in_=fc0w_in[k * P : (k + 1) * P, :],
            )
        fc0bb_sb = const.tile([P, P], F32)
        nc.sync.dma_start(out=fc0bb_sb[:], in_=fc0bb_in[:])
        fc1b_sb = const.tile([P, P], F32)
        nc.sync.dma_start(out=fc1b_sb[:], in_=fc1b_in[:])
        fc1bb_sb = const.tile([P, P], F32)
        nc.sync.dma_start(out=fc1bb_sb[:], in_=fc1bb_in[:])

        pq = psp.tile([G, P], F32, space="PSUM", tag="mm")
        for k in range(6):
            nc.tensor.matmul(
                out=pq[:],
                lhsT=qe_sb[:, k * G : (k + 1) * G],
                rhs=fc0w_sb[:, k * P : (k + 1) * P],
                start=(k == 0),
                stop=(k == 5),
            )
        qtmp = epi.tile([G, P], F32, tag="qtmp")
        nc.vector.tensor_tensor(
            out=qtmp[:], in0=pq[:], in1=fc0bb_sb[:G, :], op=mybir.AluOpType.add
        )
        qrelu = epi.tile([G, P], F32, tag="qrelu")
        nc.scalar.activation(
            out=qrelu[:], in_=qtmp[:], func=mybir.ActivationFunctionType.Relu
        )
        pqt = pst.tile([P, G], F32, space="PSUM", tag="pt")
        nc.tensor.transpose(out=pqt[:], in_=qrelu[:], identity=ident[:G, :G])
        qT = epi.tile([P, G], F32, tag="qT")
        nc.scalar.copy(out=qT[:], in_=pqt[:])
        pqq = psp.tile([G, P], F32, space="PSUM", tag="mm")
        nc.tensor.matmul(
            out=pqq[:], lhsT=qT[:], rhs=fc1b_sb[:], start=True, stop=True
        )
        qq_sb = epi.tile([G, P], FDT, tag="qq_sb")
        nc.vector.tensor_tensor(
            out=qq_sb[:], in0=pqq[:], in1=fc1bb_sb[:G, :], op=mybir.AluOpType.add
        )
        nc.sync.dma_start(out=qq_d[:], in_=qq_sb[:])

        # resident own-slice h~ buffers (self-loop term source), layer parity
        hs_keep = [
            const.tile([P, SLOT_ROWS], FDT, tag=f"hsk{i}", name=f"hsk{i}")
            for i in range(2)
        ]

        def allgather_block(l, j):
            r0 = j * SPB * P
            r1 = (j + 1) * SPB * P
            nc.gpsimd.collective_compute(
                "AllGather",
                mybir.AluOpType.bypass,
                replica_groups=[list(range(NCORES))],
                ins=[cc_in[r0:r1].opt()],
                outs=[tables[l][j * NCORES * SPB * P : (j + 1) * NCORES * SPB * P].opt()],
            )

        # ---- layer 0 production: h~0 = dinv * (x @ W0), per AG block
        for j in range(NBLK):
            xb = xp.tile([P, SPB * P], F32, tag="xb")
            nc.sync.dma_start(out=xb[:], in_=xT[:, j * SPB * P : (j + 1) * SPB * P])
            for i in range(SPB):
                s = j * SPB + i
                pp = psp.tile([P, P], F32, space="PSUM", tag="mm")
                nc.tensor.matmul(
                    out=pp[:],
                    lhsT=xb[:, i * P : (i + 1) * P],
                    rhs=W_sb[0][:],
                    start=True,
                    stop=True,
                )
                hs = hs_keep[0][:, s * P : (s + 1) * P]
                nc.scalar.activation(
                    out=hs,
                    in_=pp[:],
                    func=mybir.ActivationFunctionType.Copy,
                    scale=dinv_sb[:, s : s + 1],
                )
                nc.sync.dma_start(out=cc_in[s * P : (s + 1) * P, :], in_=hs)
            allgather_block(0, j)

        # multi-queue gather helper: <=8-chunk single-packet calls, round-robin
        # over the 4 SWDGE queues (queues generate descriptors in parallel)
        qctr = [0]

        def gather(dst_tile, dst_off, src_ap, idx_plane, col_base, nchunks):
            k = 0
            while k < nchunks:
                nb = min(8, nchunks - k)
                nc.gpsimd.dma_gather(
                    out_ap=dst_tile[
                        :, (dst_off + k) * P : (dst_off + k + nb) * P
                    ].rearrange("p (k q) -> p k q", q=P),
                    in_ap=src_ap,
                    idxs_ap=idx_plane[:, (col_base + k) * 8 : (col_base + k + nb) * 8],
                    num_idxs=nb * P,
                    num_idxs_reg=nb * P,
                    elem_size=P,
                    single_packet=True,
                    queue_num=qctr[0] % 4,
                )
                qctr[0] += 1
                k += nb

        # ---- qq rows by graph id (overlaps with AG0)
        qg_all = const.tile([P, TPC * P], FDT)
        gather(qg_all, 0, qq_d[:], gidq_sb, 0, TPC)

        # ---- 3 aggregation layers
        for l in range(3):
            table = tables[l]
            for gs, col0, lo_tot, hi_tot in grp_info:
                glo = gp.tile([P, GHMAX * P], FDT, tag="glo")
                if lo_tot:
                    gather(glo, 0, table[0:HALF], idx_sb, col0, lo_tot)
                ghi = gp.tile([P, GHMAX * P], FDT, tag="ghi")
                if hi_tot:
                    gather(ghi, 0, table[HALF:NT], idx_sb, col0 + lo_tot, hi_tot)
                ntot = lo_tot + hi_tot
                oh = ohp.tile([P, NCHG * P], FDT, tag="oh")
                if ntot:
                    nc.vector.tensor_tensor(
                        out=oh[:, : ntot * P].rearrange("p (k q) -> p k q", q=P),
                        in0=dstin_sb[:, col0 : col0 + ntot].to_broadcast([P, ntot, P]),
                        in1=iota_sb[:, : ntot * P].rearrange("p (k q) -> p k q", q=P),
                        op=mybir.AluOpType.is_equal,
                    )
                for s in gs:
                    nlo = int(nch[s, 0])
                    nhi = int(nch[s, 1])
                    lo_off = int(slot_base[s, 0]) - col0
                    hi_off = int(slot_base[s, 1]) - col0
                    lo_rel = lo_off  # within glo
                    hi_rel = hi_off - lo_tot  # within ghi
                    ps = psagg.tile([P, P], F32, space="PSUM", tag="agg")
                    for k in range(nlo):
                        nc.tensor.matmul(
                            out=ps[:],
                            lhsT=oh[:, (lo_off + k) * P : (lo_off + k + 1) * P],
                            rhs=glo[:, (lo_rel + k) * P : (lo_rel + k + 1) * P],
                            start=(k == 0),
                            stop=False,
                        )
                    for k in range(nhi):
                        nc.tensor.matmul(
                            out=ps[:],
                            lhsT=oh[:, (hi_off + k) * P : (hi_off + k + 1) * P],
                            rhs=ghi[:, (hi_rel + k) * P : (hi_rel + k + 1) * P],
                            start=False,
                            stop=False,
                        )
                    # self-loop term
                    nc.tensor.matmul(
                        out=ps[:],
                        lhsT=ident_r[:],
                        rhs=hs_keep[l % 2][:, s * P : (s + 1) * P],
                        start=False,
                        stop=False,
                    )
                    # rank-1 bias: ps += (1/dinv) outer b  (so Relu(dinv*ps) is right)
                    bp = (s % 3) * 32
                    nc.tensor.matmul(
                        out=ps[:],
                        lhsT=dinvinv_sb[bp : bp + 1, (s // 3) * P : (s // 3 + 1) * P],
                        rhs=brow_sb[bp : bp + 1, l * P : (l + 1) * P],
                        start=False,
                        stop=True,
                    )
                    hrelu = epi.tile([P, P], F32, tag="hrelu")
                    nc.scalar.activation(
                        out=hrelu[:],
                        in_=ps[:],
                        func=mybir.ActivationFunctionType.Relu,
                        scale=dinv_sb[:, s : s + 1],
                    )
                    pt = pst.tile([P, P], F32, space="PSUM", tag="pt")
                    nc.tensor.transpose(out=pt[:], in_=hrelu[:], identity=ident[:])
                    hT = epi.tile([P, P], F32, tag="hT")
                    nc.vector.tensor_copy(out=hT[:], in_=pt[:])
                    if l < 2:
                        pp2 = psp.tile([P, P], F32, space="PSUM", tag="mm")
                        nc.tensor.matmul(
                            out=pp2[:], lhsT=hT[:], rhs=W_sb[l + 1][:],
                            start=True, stop=True,
                        )
                        hs2 = hs_keep[(l + 1) % 2][:, s * P : (s + 1) * P]
                        nc.scalar.activation(
                            out=hs2,
                            in_=pp2[:],
                            func=mybir.ActivationFunctionType.Copy,
                            scale=dinv_sb[:, s : s + 1],
                        )
                        nc.sync.dma_start(
                            out=cc_in[s * P : (s + 1) * P, :], in_=hs2
                        )
                        if (s + 1) % SPB == 0:
                            allgather_block(l + 1, s // SPB)
                    else:
                        # MLP head: out = relu(h3@fc1a + qq[gid]) @ fc2 + fc2_b
                        pm = psp.tile([P, P], F32, space="PSUM", tag="mm")
                        nc.tensor.matmul(
                            out=pm[:], lhsT=hT[:], rhs=fc1a_sb[:],
                            start=True, stop=True,
                        )
                        u = epi.tile([P, P], F32, tag="u")
                        nc.vector.tensor_tensor(
                            out=u[:],
                            in0=pm[:],
                            in1=qg_all[:, s * P : (s + 1) * P],
                            op=mybir.AluOpType.add,
                        )
                        ur = epi.tile([P, P], F32, tag="ur")
                        nc.scalar.activation(
                            out=ur[:], in_=u[:],
                            func=mybir.ActivationFunctionType.Relu,
                        )
                        pt2 = pst.tile([P, P], F32, space="PSUM", tag="pt")
                        nc.tensor.transpose(out=pt2[:], in_=ur[:], identity=ident[:])
                        uT = epi.tile([P, P], F32, tag="uT")
                        nc.vector.tensor_copy(out=uT[:], in_=pt2[:])
                        po = psp.tile([P, OUTC], F32, space="PSUM", tag="mm")
                        nc.tensor.matmul(
                            out=po[:], lhsT=uT[:], rhs=fc2w_sb[:],
                            start=True, stop=False,
                        )
                        nc.tensor.matmul(
                            out=po[:], lhsT=ones_sb[:], rhs=fc2bb_sb[0:1, :],
                            start=False, stop=True,
                        )
                        ob = epi.tile([P, OUTC], F32, tag="ob")
                        nc.vector.tensor_copy(out=ob[:], in_=po[:])
                        nc.sync.dma_start(
                            out=out_d[s * P : (s + 1) * P, :], in_=ob[:]
                        )
    nc.compile()
    return nc


# ---------------------------------------------------------------- interface
_CACHE = {}


def kernel(**inputs):
    trace = bool(int(os.environ.get("GCN_TRACE", "0")))
    if trace:
        _install_axon_prof()
    from concourse.bass_utils import run_bass_kernel_spmd

    x = np.ascontiguousarray(np.asarray(inputs["x"], dtype=np.float32))
    qe = np.asarray(inputs["question_embedding"], dtype=np.float32)
    pp = preprocess(inputs["edge_index"], inputs["batch"])
    nch = pp["nch"]

    key = tuple(nch.flatten().tolist())
    if key not in _CACHE:
        _CACHE[key] = build_program(nch, pp["grp_info"], pp["slot_base"], pp["TCH"])
    nc = _CACHE[key]

    NCHG = max(lo + hi for _, _, lo, hi in pp["grp_info"])

    W = [np.asarray(inputs[f"W{i}"], np.float32) for i in range(3)]
    b = [np.asarray(inputs[f"b{i}"], np.float32) for i in range(3)]
    fc0_w = np.asarray(inputs["fc0_w"], np.float32)
    fc0_b = np.asarray(inputs["fc0_b"], np.float32)
    fc1_w = np.asarray(inputs["fc1_w"], np.float32)
    fc1_b = np.asarray(inputs["fc1_b"], np.float32)
    fc2_w = np.asarray(inputs["fc2_w"], np.float32)
    fc2_b = np.asarray(inputs["fc2_b"], np.float32)

    iota = np.broadcast_to(np.arange(P, dtype=np.float32), (P, P))
    iota_rep = np.ascontiguousarray(np.tile(iota, (1, NCHG)).astype(BF16))
    common = {
        "iota_in": iota_rep,
        "W0": W[0],
        "W1": W[1],
        "W2": W[2],
        "brow": np.broadcast_to(np.concatenate(b), (P, 3 * P)).astype(BF16).copy(),
        "qeT": np.ascontiguousarray(qe.T),
        "fc0w": fc0_w,
        "fc0bb": np.broadcast_to(fc0_b, (P, P)).copy(),
        "fc1a": np.ascontiguousarray(fc1_w[:P]),
        "fc1b": np.ascontiguousarray(fc1_w[P:]),
        "fc1bb": np.broadcast_to(fc1_b, (P, P)).copy(),
        "fc2w": fc2_w,
        "fc2bb": np.broadcast_to(fc2_b, (P, OUTC)).copy(),
        "ones_in": np.ones((1, P), np.float32),
    }

    in_maps = []
    for c in range(NCORES):
        xTc = np.zeros((P, SLOT_ROWS), dtype=np.float32)
        valid = pp["node_perm"][c] >= 0
        xTc[:, valid] = x[pp["node_perm"][c][valid]].T
        m = dict(common)
        m["xT"] = xTc
        m["idx_in"] = np.ascontiguousarray(pp["idx_T"][c])
        m["dstin"] = np.ascontiguousarray(pp["dstin_T"][c].astype(BF16))
        m["dinv_in"] = np.ascontiguousarray(pp["dinv_slot"][c])
        m["dinvinv_in"] = np.ascontiguousarray(pp["dinvinv_slot"][c].astype(BF16))
        m["gidq_in"] = np.ascontiguousarray(pp["gidq_T"][c])
        in_maps.append(m)

    res = run_bass_kernel_spmd(
        nc,
        in_maps,
        list(range(NCORES)),
        trace=trace,
    )
    kernel.last_result = res

    out = np.zeros((N, OUTC), dtype=np.float32)
    for c in range(NCORES):
        valid = pp["node_perm"][c] >= 0
        out[pp["node_perm"][c][valid]] = res.results[c]["out"][valid]
    return out
